# revision 1
# baseline (speedup 1.0000x reference)
"""Dense MLP kernel for Trainium2: y = inputs @ kernel + bias.

Full shapes: inputs (4, 2048, 4096) f32, kernel (4096, 16384) f32,
bias (16384,) f32 -> y (4, 2048, 16384) f32.

Strategy: tensor-parallel over the output feature dim F=16384, split 8
ways (2048 features per core). Each core receives the full activations
(pre-transposed on the host to [d, tok] tile layout, shared across all
cores) plus its weight slice, computes Y_c = X @ W_c + bias_c, and the
host concatenates the per-core outputs along F. No device collectives.

Numerics: float32r matmuls (fp32 operands, relaxed-precision PE mode at
full 1-cycle/row rate) with fp32 PSUM accumulation: measured ~1.5e-4
relative error vs fp64 at full scale, ~15x better than bf16, at the
same speed.

Per-core program: the weight slice is processed in two f-halves of
[4096, 1024] fp32 (128KB/partition, SBUF-resident); activations are
streamed twice as [4096, 128]-token column tiles (stationary operand);
each 128-token tile accumulates 32 k-subtiles into 2 PSUM banks (one
per 512-wide feature chunk). Bias is added during the PSUM->SBUF
eviction on the vector engine. Measured steady-state ~2.28 ms/core on
8 concurrently-running cores, equal to the pure-matmul pipeline floor
(the 8192 N=512 matmuls per core run at the sustained-load PE rate).
"""

import numpy as np

# Problem constants (hardcoded per the task contract).
B, S, D, F = 4, 2048, 4096, 16384
T = B * S  # 8192 tokens
P = 128
NCORES = 8

FD = 512  # matmul free dim (one fp32 PSUM bank)
FC = F // NCORES  # 2048 features per core
KS = D // P  # 32 k-subtiles
NTT = T // P  # 64 token tiles
NH = 2  # weight halves per core
FCH = FC // NH  # 1024
NFC_H = FCH // FD  # 2 feature chunks per half

_COMPILED = None


def _build(repeat=1):
    import concourse.bacc as bacc
    import concourse.mybir as mybir
    import concourse.tile as tile

    DT = mybir.dt.float32r
    nc = bacc.Bacc("TRN2", target_bir_lowering=False, debug=False)

    xt = nc.dram_tensor("xt", (P, NTT, KS, P), DT, kind="ExternalInput")
    w = nc.dram_tensor("w", (P, KS, FC), DT, kind="ExternalInput")
    bias = nc.dram_tensor("bias", (P, FC), mybir.dt.float32, kind="ExternalInput")
    y = nc.dram_tensor(
        "y", (P, NTT, NH, NFC_H, FD), mybir.dt.float32, kind="ExternalOutput"
    )

    with tile.TileContext(nc) as tc:
        with (
            tc.tile_pool(name="wpool", bufs=1) as wpool,
            tc.tile_pool(name="bpool", bufs=1) as bpool,
            tc.tile_pool(name="xpool", bufs=2) as xpool,
            tc.tile_pool(name="opool", bufs=4) as opool,
            tc.tile_pool(name="pspool", bufs=8, space="PSUM") as pspool,
        ):
            def body():
                b_sb = bpool.tile([P, FC], mybir.dt.float32, name="b_sb")
                nc.sync.dma_start(out=b_sb[:], in_=bias[:, :])
                for h in range(NH):
                    w_sb = wpool.tile([P, KS, FCH], DT, name="w_sb")
                    gs = KS // 8
                    for g in range(8):
                        nc.sync.dma_start(
                            out=w_sb[:, g * gs : (g + 1) * gs, :],
                            in_=w[:, g * gs : (g + 1) * gs, h * FCH : (h + 1) * FCH],
                        )
                    for tt in range(NTT):
                        x_sb = xpool.tile([P, KS, P], DT, name="x_sb")
                        nc.sync.dma_start(out=x_sb[:], in_=xt[:, tt, :, :])
                        psums = [
                            pspool.tile([P, FD], mybir.dt.float32, name="ps")
                            for _ in range(NFC_H)
                        ]
                        for ks in range(KS):
                            for fc in range(NFC_H):
                                nc.tensor.matmul(
                                    psums[fc][:],
                                    lhsT=x_sb[:, ks, :],
                                    rhs=w_sb[:, ks, fc * FD : (fc + 1) * FD],
                                    start=(ks == 0),
                                    stop=(ks == KS - 1),
                                )
                        for fc in range(NFC_H):
                            o_sb = opool.tile([P, FD], mybir.dt.float32, name="o_sb")
                            nc.vector.tensor_tensor(
                                out=o_sb[:],
                                in0=psums[fc][:],
                                in1=b_sb[
                                    :, h * FCH + fc * FD : h * FCH + (fc + 1) * FD
                                ],
                                op=mybir.AluOpType.add,
                            )
                            nc.sync.dma_start(out=y[:, tt, h, fc, :], in_=o_sb[:])

            if repeat == 1:
                body()
            else:
                with tc.For_i(0, repeat, 1):
                    body()

    nc.compile()
    return nc


def _get_compiled():
    global _COMPILED
    if _COMPILED is None:
        _COMPILED = _build()
    return _COMPILED


def prep_inputs(inputs, kernel, bias):
    x32 = np.ascontiguousarray(
        np.asarray(inputs, dtype=np.float32).reshape(T, D)
    )
    # xt[p, tt, ks, t] = X[tt*128+t, ks*128+p]
    xt_host = np.ascontiguousarray(x32.reshape(NTT, P, KS, P).transpose(3, 0, 2, 1))
    w32 = np.asarray(kernel, dtype=np.float32)
    # w[p, ks, f] = W[ks*128+p, f]
    w_host = np.ascontiguousarray(w32.reshape(KS, P, F).transpose(1, 0, 2))
    b32 = np.asarray(bias, dtype=np.float32)
    in_maps = []
    for c in range(NCORES):
        in_maps.append(
            {
                "xt": xt_host,
                "w": np.ascontiguousarray(w_host[:, :, c * FC : (c + 1) * FC]),
                "bias": np.ascontiguousarray(
                    np.broadcast_to(b32[c * FC : (c + 1) * FC], (P, FC))
                ),
            }
        )
    return in_maps


def gather(results):
    out = np.empty((T, F), dtype=np.float32)
    for c in range(NCORES):
        y_c = results[c]["y"]  # [P, NTT, NH, NFC_H, FD]
        out[:, c * FC : (c + 1) * FC] = (
            y_c.reshape(P, NTT, FC).transpose(1, 0, 2).reshape(T, FC)
        )
    return out.reshape(B, S, F)


def kernel(**inputs):
    from concourse import bass_utils

    nc = _get_compiled()
    in_maps = prep_inputs(inputs["inputs"], inputs["kernel"], inputs["bias"])
    last_err = None
    for _attempt in range(3):
        try:
            res = bass_utils.run_bass_kernel_spmd(
                nc, in_maps, core_ids=list(range(NCORES)), trace=False
            )
            return gather(res.results)
        except Exception as e:  # transient NRT/axon errors observed ~rarely
            last_err = e
    raise last_err



# revision 3
# speedup vs baseline: 1.5788x; 1.5788x over previous
"""Dense MLP kernel for Trainium2: y = inputs @ kernel + bias.

Full shapes: inputs (4, 2048, 4096) f32, kernel (4096, 16384) f32,
bias (16384,) f32 -> y (4, 2048, 16384) f32.

Strategy: tensor-parallel over the output feature dim F=16384, split 8
ways (2048 features per core). Each core receives the full activations
(pre-transposed on the host to [d, tok] tile layout and cast to bf16,
shared across all cores) plus its bf16 weight slice, computes
Y_c = X @ W_c + bias_c in fp32 PSUM, and the host concatenates the
per-core outputs along F. No device collectives.

Numerics: bf16 operands with fp32 PSUM accumulation: measured 2.35e-3
L2 relative error vs fp64 at full scale (gate is 2e-2).

Per-core program (single pass, W fully SBUF-resident):
- W slice [128k x 32ks x 2048f] bf16 (128KB/partition) loaded once.
- Activations streamed once as 64 token tiles [128k x 32ks x 128t] bf16
  (double-buffered pool, sync-engine DMA ring).
- Per token tile: 128 matmuls (32 k-subtiles x 4 f-chunks of 512);
  each stationary x tile feeds 4 N=512 matmuls into 4 PSUM banks
  (amortizes the bf16 embedded weight-load); PSUM pool rotates 2 sets
  so tt+1 matmuls overlap tt evictions.
- Bias added during PSUM->SBUF eviction on the vector engine; y stores
  issued on the scalar-engine DMA ring so store waits never block the
  x-load prefetch FIFO.

Measured 2.26 ms steady-state on 8 concurrent cores (repeat-loop
slope), vs 2.27 ms baseline; the pure-matmul 8-core sustained ceiling
measured for this instruction mix is ~2.2 ms (chip-level power/clock
throttle; single-core runs ~20% faster per-matmul than 8-core).
"""

import numpy as np

B, S, D, F = 4, 2048, 4096, 16384
T = B * S
P = 128
NCORES = 8

FD = 512
FC = F // NCORES  # 2048
KS = D // P  # 32
NTT = T // P  # 64
NFC = FC // FD  # 4

_COMPILED = None


def _build(repeat=1):
    import concourse.bacc as bacc
    import concourse.mybir as mybir
    import concourse.tile as tile

    DT = mybir.dt.bfloat16
    nc = bacc.Bacc("TRN2", target_bir_lowering=False, debug=False)

    xt = nc.dram_tensor("xt", (P, NTT, KS, P), DT, kind="ExternalInput")
    w = nc.dram_tensor("w", (P, KS, FC), DT, kind="ExternalInput")
    bias = nc.dram_tensor("bias", (P, FC), mybir.dt.float32, kind="ExternalInput")
    y = nc.dram_tensor(
        "y", (P, NTT, NFC, FD), mybir.dt.float32, kind="ExternalOutput"
    )

    with tile.TileContext(nc) as tc:
        with (
            tc.tile_pool(name="wpool", bufs=1) as wpool,
            tc.tile_pool(name="bpool", bufs=1) as bpool,
            tc.tile_pool(name="xpool", bufs=4) as xpool,
            tc.tile_pool(name="opool", bufs=8) as opool,
            tc.tile_pool(name="pspool", bufs=2, space="PSUM") as pspool,
        ):
            def body():
                b_sb = bpool.tile([P, FC], mybir.dt.float32, name="b_sb")
                nc.scalar.dma_start(out=b_sb[:], in_=bias[:, :])
                w_sb = wpool.tile([P, KS, FC], DT, name="w_sb")
                gs = KS // 8
                for g in range(8):
                    nc.sync.dma_start(
                        out=w_sb[:, g * gs : (g + 1) * gs, :],
                        in_=w[:, g * gs : (g + 1) * gs, :],
                    )
                for tt in range(NTT):
                    x_sb = xpool.tile([P, KS, P], DT, name="x_sb")
                    nc.sync.dma_start(out=x_sb[:], in_=xt[:, tt, :, :])
                    psums = [
                        pspool.tile([P, FD], mybir.dt.float32, name=f"ps{i}")
                        for i in range(NFC)
                    ]
                    for ks in range(KS):
                        for fc in range(NFC):
                            nc.tensor.matmul(
                                psums[fc][:],
                                lhsT=x_sb[:, ks, :],
                                rhs=w_sb[:, ks, fc * FD : (fc + 1) * FD],
                                start=(ks == 0),
                                stop=(ks == KS - 1),
                            )
                    for fc in range(NFC):
                        o_sb = opool.tile([P, FD], mybir.dt.float32, name="o_sb")
                        nc.vector.tensor_tensor(
                            out=o_sb[:],
                            in0=psums[fc][:],
                            in1=b_sb[:, fc * FD : (fc + 1) * FD],
                            op=mybir.AluOpType.add,
                        )
                        nc.scalar.dma_start(out=y[:, tt, fc, :], in_=o_sb[:])

            if repeat == 1:
                body()
            else:
                with tc.For_i(0, repeat, 1):
                    body()

    nc.compile()
    return nc


def _get_compiled():
    global _COMPILED
    if _COMPILED is None:
        _COMPILED = _build()
    return _COMPILED


def prep_inputs(inputs, kernel, bias):
    import ml_dtypes

    x32 = np.ascontiguousarray(np.asarray(inputs, dtype=np.float32).reshape(T, D))
    xt_host = np.ascontiguousarray(
        x32.reshape(NTT, P, KS, P).transpose(3, 0, 2, 1).astype(ml_dtypes.bfloat16)
    )
    w32 = np.asarray(kernel, dtype=np.float32)
    w_host = np.ascontiguousarray(
        w32.reshape(KS, P, F).transpose(1, 0, 2).astype(ml_dtypes.bfloat16)
    )
    b32 = np.asarray(bias, dtype=np.float32)
    in_maps = []
    for c in range(NCORES):
        in_maps.append(
            {
                "xt": xt_host,
                "w": np.ascontiguousarray(w_host[:, :, c * FC : (c + 1) * FC]),
                "bias": np.ascontiguousarray(
                    np.broadcast_to(b32[c * FC : (c + 1) * FC], (P, FC))
                ),
            }
        )
    return in_maps


def gather(results):
    out = np.empty((T, F), dtype=np.float32)
    for c in range(NCORES):
        y_c = results[c]["y"]  # [P, NTT, NFC, FD]
        out[:, c * FC : (c + 1) * FC] = (
            y_c.reshape(P, NTT, FC).transpose(1, 0, 2).reshape(T, FC)
        )
    return out.reshape(B, S, F)


def kernel(**inputs):
    from concourse import bass_utils

    nc = _get_compiled()
    in_maps = prep_inputs(inputs["inputs"], inputs["kernel"], inputs["bias"])
    last_err = None
    for _attempt in range(3):
        try:
            res = bass_utils.run_bass_kernel_spmd(
                nc, in_maps, core_ids=list(range(NCORES)), trace=False
            )
            return gather(res.results)
        except Exception as e:
            last_err = e
    raise last_err
